# revision 12
# baseline (speedup 1.0000x reference)
"""Multi-head attention (B=8, P=1024, D=768, H=12) on 8 TRN2 NeuronCores, v2.

Strategy: pure data parallelism — batch element b runs on core b. All matmuls
bf16 on the PE (fp8 fails the 2e-2 gate: attention-score sensitivity).
Relative to v1: the ScalarEngine runs ONLY exp (softmax reciprocal moved to
DVE InstReciprocal; wa-transpose matmuls and their PSUM->SBUF copies removed
by emitting wa transposed and letting the host transpose back; per-head
normalization via an SBUF broadcast-DMA of the reciprocal row + full-speed
bf16 DVE multiplies instead of selector matmuls through PSUM), stg
evacuations moved to GPSIMD, and v/proj matmuls use full 768-wide moving
operands.
"""

import numpy as np
from contextlib import ExitStack

import bass_rust
import concourse.bass as bass
import concourse.tile as tile
from concourse import mybir
from concourse import bass2jax

B, P, D = 8, 1024, 768
H = 12
HD = D // H            # 64
SCALE = HD ** -0.5     # 0.125
N_CORES = 8
KT = D // 128          # 6 contraction tiles over d
QT = P // 128          # 8 tiles over sequence
BF = mybir.dt.bfloat16
F32 = mybir.dt.float32
NP_BF16 = np.dtype(mybir.dt.np(BF))

IN_NAMES = ["xT", "wqk", "wv", "wp", "bqk", "bv", "bp"]
OUT_NAMES = ["out", "waT"]


def _split_excess_waits(nc, max_waits=1):
    """Walrus rejects instructions with more than one sync wait; hoist excess
    waits onto same-engine no-ops inserted before the overloaded instruction."""
    ctr = 0
    for bb in nc.main_func.blocks:
        newlist = []
        dirty = False
        for inst in bb.instructions:
            si = inst.sync_info
            waits = list(si.on_wait) if (si is not None and si.on_wait) else []
            if len(waits) > max_waits:
                excess, keep = waits[:-max_waits], waits[-max_waits:]
                for i in range(0, len(excess), max_waits):
                    chunk = excess[i : i + max_waits]
                    nop = bass_rust.InstNoOp(name=f"WSPILL-{ctr}")
                    ctr += 1
                    nop.engine = inst.engine
                    nop.sync_info = bass_rust.SyncInfo(on_wait=chunk, on_update=[])
                    newlist.append(nop)
                inst.sync_info = bass_rust.SyncInfo(
                    on_wait=keep, on_update=list(si.on_update or [])
                )
                dirty = True
            newlist.append(inst)
        if dirty:
            bb.instructions = newlist
    return ctr


def _bcast_dram(dram_ap, parts):
    """Partition-stride-0 DMA source view of a 1-D DRAM tensor: [n] -> [parts, n]."""
    return bass.AP(
        tensor=dram_ap.tensor,
        offset=dram_ap.offset,
        ap=[[0, parts]] + list(dram_ap.ap),
    )


def _bcast_row(row_ap, parts):
    """SBUF one-row AP -> [parts, n] via a free-dim stride-0 middle dim."""
    ap = list(row_ap.ap)
    return bass.AP(
        tensor=row_ap.tensor,
        offset=row_ap.offset,
        ap=[ap[0], [0, parts]] + ap[1:],
    )


def build_nc(split_waits=True, max_phase=6, loop_n=None, unroll=1, probe=None):
    nc = bass.Bass(target_bir_lowering=False)

    xT_e = nc.declare_dram_parameter("xT", [128, KT, P], BF, isOutput=False)
    wqk_e = nc.declare_dram_parameter("wqk", [128, KT, 2 * D], BF, isOutput=False)
    wv_e = nc.declare_dram_parameter("wv", [128, KT, D], BF, isOutput=False)
    wp_e = nc.declare_dram_parameter("wp", [128, KT, D], BF, isOutput=False)
    bqk_e = nc.declare_dram_parameter("bqk", [128, 2 * D // 128], F32, isOutput=False)
    bv_e = nc.declare_dram_parameter("bv", [D], F32, isOutput=False)
    bp_e = nc.declare_dram_parameter("bp", [D], F32, isOutput=False)
    out_e = nc.declare_dram_parameter("out", [P, D], BF, isOutput=True)
    waT_e = nc.declare_dram_parameter("waT", [D, P], BF, isOutput=True)

    EXP = mybir.ActivationFunctionType.Exp

    with tile.TileContext(nc) as tc, ExitStack() as ctx:
        if loop_n is not None:
            ctx.enter_context(tc.For_i(0, loop_n, 1))
        const = ctx.enter_context(tc.tile_pool(name="const", bufs=1))
        qkp = ctx.enter_context(tc.tile_pool(name="qkp", bufs=1))
        vxp = ctx.enter_context(tc.tile_pool(name="vxp", bufs=1))
        wtp = ctx.enter_context(tc.tile_pool(name="wtp", bufs=1))
        ptp = ctx.enter_context(tc.tile_pool(name="ptp", bufs=12))
        stgp = ctx.enter_context(tc.tile_pool(name="stgp", bufs=1))
        rbp = ctx.enter_context(tc.tile_pool(name="rbp", bufs=1))
        outp = ctx.enter_context(tc.tile_pool(name="outp", bufs=4))
        psum = ctx.enter_context(tc.tile_pool(name="psum", bufs=4, space="PSUM"))

        def alloc_consts():
            # ---- constant loads: DMA order = first-use order, chunked ------
            C = {}
            C["xT"] = const.tile([128, KT, P], BF, tag="xT", name="xT")
            C["wqk"] = const.tile([128, KT, 2 * D], BF, tag="wqk", name="wqk")
            C["wv"] = const.tile([128, KT, D], BF, tag="wv", name="wv")
            C["wp"] = const.tile([128, KT, D], BF, tag="wp", name="wp")
            C["bqk"] = const.tile([128, 2 * D // 128], F32, tag="bqk", name="bqk")
            C["bvb"] = const.tile([128, D], F32, tag="bvb", name="bvb")
            C["bpb"] = const.tile([128, D], F32, tag="bpb", name="bpb")
            for c in range(3):
                nc.sync.dma_start(out=C["xT"][:, 2 * c : 2 * c + 2, :], in_=xT_e[:, 2 * c : 2 * c + 2, :])
                nc.sync.dma_start(out=C["wqk"][:, 2 * c : 2 * c + 2, :], in_=wqk_e[:, 2 * c : 2 * c + 2, :])
            for c in range(2):
                nc.sync.dma_start(out=C["wv"][:, 3 * c : 3 * c + 3, :], in_=wv_e[:, 3 * c : 3 * c + 3, :])
            nc.sync.dma_start(out=C["bqk"], in_=bqk_e[:])
            nc.sync.dma_start(out=C["bvb"], in_=_bcast_dram(bv_e[:], 128))
            for c in range(2):
                nc.sync.dma_start(out=C["wp"][:, 3 * c : 3 * c + 3, :], in_=wp_e[:, 3 * c : 3 * c + 3, :])
            nc.sync.dma_start(out=C["bpb"], in_=_bcast_dram(bp_e[:], 128))
            return C

        def alloc_qkT():
            return [qkp.tile([128, P], BF, tag=f"qkT{m}", name=f"qkT{m}") for m in range(2 * D // 128)]

        def emit_qkT_c(C, qkTl, ms):
            for m in ms:
                ps = psum.tile([128, 1024], F32, tag="ps", name="ps")
                for k in range(KT):
                    for j in range(2):
                        nc.tensor.matmul(
                            ps[:, j * 512 : (j + 1) * 512],
                            lhsT=C["wqk"][:, k, m * 128 : (m + 1) * 128],
                            rhs=C["xT"][:, k, j * 512 : (j + 1) * 512],
                            start=(k == 0),
                            stop=(k == KT - 1),
                        )
                nc.vector.tensor_scalar_add(qkTl[m], ps, C["bqk"][:, m : m + 1])

        Ccur = qkTcur = None
        for _it in range(unroll):
            first = Ccur is None
            if first:
                Ccur = alloc_consts()
            C = Ccur
            xT, wqk, wv, wp = C["xT"], C["wqk"], C["wv"], C["wp"]
            bqk, bvb, bpb = C["bqk"], C["bvb"], C["bpb"]

            if max_phase < 2:
                for qt in range(QT):
                    nc.sync.dma_start(out=out_e[qt * 128 : (qt + 1) * 128, :], in_=xT[:, 0, 0:D])
                for pr in range(6):
                    nc.gpsimd.dma_start(out=waT_e[pr * 128 : (pr + 1) * 128, :], in_=xT[:, 1, :])
                continue

            # ---- phase 1: qT / kT = (w_qk)^T @ x^T  [feature-major] --------
            # qkT[m] rows = features m*128..; m 0..5 -> q, 6..11 -> k.
            # The first two tiles of iteration i+1 are emitted in iteration
            # i's tail (software pipelining); remaining pairs interleave.
            if first:
                qkTcur = alloc_qkT()
            qkT = qkTcur

            def emit_qkT(ms):
                emit_qkT_c(C, qkT, ms)

            first_ms = [0, 6] if (probe is None and max_phase >= 3) else ([] if probe == 'v' else list(range(12)))
            if first:
                emit_qkT(first_ms)

            # ---- phase 2: v natural [seq-major] with ones column ------------
            vext = [vxp.tile([128, H, HD + 1], BF, tag=f"vext{p}", name=f"vext{p}") for p in range(QT)]
            for p in range(QT if probe != 'qkT' else 0):
                nc.vector.memset(vext[p][:, :, HD : HD + 1], 1.0)
                ps = psum.tile([128, 1024], F32, tag="ps", name="ps")
                for k in range(KT):
                    for (c0, cw) in ((0, 512), (512, 256)):
                        nc.tensor.matmul(
                            ps[:, c0 : c0 + cw],
                            lhsT=xT[:, k, p * 128 : (p + 1) * 128],
                            rhs=wv[:, k, c0 : c0 + cw],
                            start=(k == 0),
                            stop=(k == KT - 1),
                        )
                nc.vector.tensor_add(
                    vext[p][:, :, 0:HD],
                    ps[:, 0:D].rearrange("p (h d) -> p h d", d=HD),
                    bvb.rearrange("p (h d) -> p h d", d=HD),
                )

            # ---- phase 3: attention per head pair ---------------------------
            # S^T tiles [k-positions, q]; exp on ScalarE (scale folded); AV
            # lags one k-tile. Per-pair AV accumulators [65,1024] hold the
            # softmax denominator in row 64 (ones column of vext).
            waTp = [wtp.tile([128, P], BF, tag=f"waTp{p}", name=f"waTp{p}") for p in range(H // 2)]
            densA = stgp.tile([8, P], BF, tag="densA", name="densA")
            densB1 = stgp.tile([2, P], BF, tag="densB1", name="densB1")
            densB2 = stgp.tile([2, P], BF, tag="densB2", name="densB2")
            recipA = stgp.tile([8, P], BF, tag="recipA", name="recipA")
            recipB1 = stgp.tile([2, P], BF, tag="recipB1", name="recipB1")
            recipB2 = stgp.tile([2, P], BF, tag="recipB2", name="recipB2")
            rb = [rbp.tile([HD, P], BF, tag=f"rb{h}", name=f"rb{h}") for h in range(H)]
            stg_tiles = {}

            def emit_recip(grp):
                # 1/denominator on DVE (bf16 out), then per-head broadcast.
                # Reciprocal partition offsets must be 0/32-aligned, so each
                # batch lives at partition 0 of its own tile.
                dens, recip, hs = {
                    0: (densA, recipA, range(0, 8)),
                    1: (densB1, recipB1, range(8, 10)),
                    2: (densB2, recipB2, range(10, 12)),
                }[grp]
                with nc.allow_low_precision(reason="softmax reciprocal to bf16"):
                    nc.vector.reciprocal(recip, dens)
                for h in hs:
                    r = recip[h - hs[0] : h - hs[0] + 1, :]
                    nc.sync.dma_start(out=rb[h], in_=_bcast_row(r, HD))

            def emit_norm(heads):
                # waTp[pair][0:64] = stg_even * rb ; odd half staged through a
                # temp then DMA-merged into partitions 64:128
                for h in heads:
                    if h % 2 == 0:
                        nc.vector.tensor_mul(
                            waTp[h // 2][0:HD, :], stg_tiles[h][0:HD, :], rb[h]
                        )
                    else:
                        wt = outp.tile([HD, P], BF, tag="wtmp", name="wtmp")
                        nc.vector.tensor_mul(wt, stg_tiles[h][0:HD, :], rb[h])
                        eng = nc.sync if h >= 10 else nc.gpsimd
                        eng.dma_start(out=waTp[h // 2][HD:128, :], in_=wt)

            def emit_waT(pr):
                nc.gpsimd.dma_start(out=waT_e[pr * 128 : (pr + 1) * 128, :], in_=waTp[pr])

            # Flat (pair, kt) stream: AV lags S/exp by AV_LAG steps and flows
            # ACROSS pair boundaries, so a new pair's S matmuls cover the old
            # pair's accumulator evacuation latency.
            NPAIR = H // 2 if max_phase >= 3 else 0
            AV_LAG = 4
            psav_all = {}
            pts = {}

            def emit_pair_tail(pr):
                heads = (2 * pr, 2 * pr + 1)
                for h in heads:
                    stg = stgp.tile([HD + 1, P], BF, tag=f"stg{h}", name=f"stg{h}")
                    # last pair: odd head's evac on the (now idle) ScalarE so
                    # both heads evacuate in parallel ahead of the tail chain
                    if h == 11:
                        nc.scalar.copy(stg, psav_all[h])
                    else:
                        nc.vector.tensor_copy(stg, psav_all[h])
                    if h < 8:
                        dtile = densA[h : h + 1, :]
                    elif h < 10:
                        dtile = densB1[h - 8 : h - 7, :]
                    else:
                        dtile = densB2[h - 10 : h - 9, :]
                    nc.sync.dma_start(out=dtile, in_=stg[HD : HD + 1, :])
                    stg_tiles[h] = stg
                if max_phase >= 4:
                    if pr == 3:
                        emit_recip(0)
                        emit_norm(range(0, 4))
                    elif pr == 4:
                        emit_recip(1)
                        emit_norm([8, 9, 4, 5, 6, 7])
                        for p2 in range(0, 5):
                            emit_waT(p2)
                    elif pr == 5:
                        emit_recip(2)
                        emit_norm([11, 10])
                        emit_waT(5)

            def emit_av(pr, kt):
                heads = (2 * pr, 2 * pr + 1)
                if kt == 0:
                    for h in heads:
                        psav_all[h] = psum.tile([HD + 1, P], F32, tag="ps", name="ps")
                for h in heads:
                    for j in range(2):
                        nc.tensor.matmul(
                            psav_all[h][:, j * 512 : (j + 1) * 512],
                            lhsT=vext[kt][:, h, :],
                            rhs=pts[(pr, kt, h)][:, j * 512 : (j + 1) * 512],
                            start=(kt == 0),
                            stop=(kt == QT - 1),
                        )
                if kt == QT - 1:
                    emit_pair_tail(pr)

            steps = [(pr, kt) for pr in range(NPAIR) for kt in range(QT)]
            for i, (pr, kt) in enumerate(steps):
                heads = (2 * pr, 2 * pr + 1)
                pss = {h: psum.tile([128, P], F32, tag="ps", name="ps") for h in heads}
                for h in heads:
                    base = (h % 2) * 64
                    for j in range(2):
                        nc.tensor.matmul(
                            pss[h][:, j * 512 : (j + 1) * 512],
                            lhsT=qkT[6 + h // 2][base : base + 64, kt * 128 : (kt + 1) * 128],
                            rhs=qkT[h // 2][base : base + 64, j * 512 : (j + 1) * 512],
                            start=True,
                            stop=True,
                        )
                    pt = ptp.tile([128, P], BF, tag="pt")
                    nc.scalar.activation(pt, pss[h], EXP, scale=SCALE)
                    pts[(pr, kt, h)] = pt
                # mid-pair qkT interleave keeps the PE warm while exp runs
                if kt == 3 and pr + 1 < NPAIR:
                    emit_qkT([pr + 1])
                if kt == 6 and pr + 1 < NPAIR:
                    emit_qkT([6 + pr + 1])
                if i >= AV_LAG:
                    emit_av(*steps[i - AV_LAG])
            for (pr, kt) in steps[len(steps) - AV_LAG :]:
                emit_av(pr, kt)

            # ---- phase 4: proj per q-tile ----------------------------------
            # First 4 q-tiles accumulate pairs 0..4 while the last pair's
            # normalization chain completes, then take pair 5 and evacuate.
            def proj_mm(ps, qt, p, start, stop):
                for (c0, cw) in ((0, 512), (512, 256)):
                    nc.tensor.matmul(
                        ps[:, c0 : c0 + cw],
                        lhsT=waTp[p][:, qt * 128 : (qt + 1) * 128],
                        rhs=wp[:, p, c0 : c0 + cw],
                        start=start,
                        stop=stop,
                    )

            def proj_fin(ps, qt):
                proj_mm(ps, qt, H // 2 - 1, False, True)
                out_sb = outp.tile([128, D], BF, tag="out_sb", name="out_sb")
                nc.vector.tensor_add(out_sb, ps[:, 0:D], bpb)
                nc.gpsimd.dma_start(out=out_e[qt * 128 : (qt + 1) * 128, :], in_=out_sb)

            Cnext = None
            if max_phase >= 6:
                pss_proj = {}
                for qt in range(2):
                    ps = psum.tile([128, 1024], F32, tag="ps", name="ps")
                    for p in range(H // 2 - 1):
                        proj_mm(ps, qt, p, p == 0, False)
                    pss_proj[qt] = ps
                # next iteration's prologue fills the PE while the last
                # pair's normalization chain completes
                if _it + 1 < unroll and probe is None and max_phase >= 6:
                    Cnext = alloc_consts()
                    qkTnext = alloc_qkT()
                    emit_qkT_c(Cnext, qkTnext, [0, 6])
                for qt in range(2):
                    proj_fin(pss_proj[qt], qt)
                for qt in range(2, QT):
                    ps = psum.tile([128, 1024], F32, tag="ps", name="ps")
                    for p in range(H // 2 - 1):
                        proj_mm(ps, qt, p, p == 0, False)
                    proj_fin(ps, qt)
            if Cnext is not None:
                Ccur, qkTcur = Cnext, qkTnext
            else:
                Ccur = qkTcur = None

            if max_phase < 6:
                for qt in range(QT):
                    nc.sync.dma_start(out=out_e[qt * 128 : (qt + 1) * 128, :], in_=qkT[0][:, 0:D])
            if max_phase < 4:
                for pr in range(6):
                    nc.gpsimd.dma_start(out=waT_e[pr * 128 : (pr + 1) * 128, :], in_=qkT[1][:, 0:P])

    if split_waits:
        _split_excess_waits(nc)
    return nc


def make_in_maps(x, w_qkv, b_qkv, w_proj, b_proj):
    """Host-side shard prep: batch element b -> core b; weights replicated."""
    xf = np.asarray(x, dtype=np.float32)
    wqkv = np.asarray(w_qkv, dtype=np.float32)
    bqkv = np.asarray(b_qkv, dtype=np.float32)
    wproj = np.asarray(w_proj, dtype=np.float32)
    bproj = np.asarray(b_proj, dtype=np.float32)

    wqk = np.ascontiguousarray(
        wqkv[:, : 2 * D].reshape(KT, 128, 2 * D).transpose(1, 0, 2)
    ).astype(NP_BF16)
    wv = np.ascontiguousarray(
        wqkv[:, 2 * D :].reshape(KT, 128, D).transpose(1, 0, 2)
    ).astype(NP_BF16)
    wp = np.ascontiguousarray(
        wproj.reshape(KT, 128, D).transpose(1, 0, 2)
    ).astype(NP_BF16)
    bqk = np.ascontiguousarray(bqkv[: 2 * D].reshape(2 * D // 128, 128).T)
    bv = np.ascontiguousarray(bqkv[2 * D :])

    in_maps = []
    for b in range(N_CORES):
        xT = np.ascontiguousarray(
            xf[b].T.reshape(KT, 128, P).transpose(1, 0, 2)
        ).astype(NP_BF16)
        in_maps.append(
            {
                "xT": xT,
                "wqk": wqk,
                "wv": wv,
                "wp": wp,
                "bqk": bqk,
                "bv": bv,
                "bp": bproj,
            }
        )
    return in_maps


_CACHE = {}


def _get_nc():
    if "nc" not in _CACHE:
        _CACHE["nc"] = build_nc()
    return _CACHE["nc"]


def run_once(in_maps, nc=None):
    if nc is None:
        nc = _get_nc()
    return bass2jax.run_bass_via_pjrt(nc, in_maps, n_cores=N_CORES)


def kernel(x, w_qkv, b_qkv, w_proj, b_proj):
    in_maps = make_in_maps(x, w_qkv, b_qkv, w_proj, b_proj)
    results = run_once(in_maps)
    out = np.stack([results[b]["out"] for b in range(N_CORES)]).astype(np.float32)
    wa = np.stack(
        [results[b]["waT"].astype(np.float32).T for b in range(N_CORES)]
    )
    return (out, wa)


# revision 14
# speedup vs baseline: 1.3774x; 1.3774x over previous
"""Multi-head attention (B=8, P=1024, D=768, H=12) on 8 TRN2 NeuronCores, v2.

Strategy: pure data parallelism — batch element b runs on core b. All matmuls
bf16 on the PE (fp8 fails the 2e-2 gate: attention-score sensitivity).
Relative to v1: the ScalarEngine runs ONLY exp (softmax reciprocal moved to
DVE InstReciprocal; wa-transpose matmuls and their PSUM->SBUF copies removed
by emitting wa transposed and letting the host transpose back; per-head
normalization via an SBUF broadcast-DMA of the reciprocal row + full-speed
bf16 DVE multiplies instead of selector matmuls through PSUM), stg
evacuations moved to GPSIMD, and v/proj matmuls use full 768-wide moving
operands.
"""

import numpy as np
from contextlib import ExitStack

import bass_rust
import concourse.bass as bass
import concourse.tile as tile
from concourse import mybir
from concourse import bass2jax

B, P, D = 8, 1024, 768
H = 12
HD = D // H            # 64
SCALE = HD ** -0.5     # 0.125
N_CORES = 8
KT = D // 128          # 6 contraction tiles over d
QT = P // 128          # 8 tiles over sequence
BF = mybir.dt.bfloat16
F32 = mybir.dt.float32
NP_BF16 = np.dtype(mybir.dt.np(BF))

IN_NAMES = ["xT", "wqk", "wv", "wp", "bqk", "bv", "bp"]
OUT_NAMES = ["out", "waT"]


def _split_excess_waits(nc, max_waits=1):
    """Walrus rejects instructions with more than one sync wait; hoist excess
    waits onto same-engine no-ops inserted before the overloaded instruction."""
    ctr = 0
    for bb in nc.main_func.blocks:
        newlist = []
        dirty = False
        for inst in bb.instructions:
            si = inst.sync_info
            waits = list(si.on_wait) if (si is not None and si.on_wait) else []
            if len(waits) > max_waits:
                excess, keep = waits[:-max_waits], waits[-max_waits:]
                for i in range(0, len(excess), max_waits):
                    chunk = excess[i : i + max_waits]
                    nop = bass_rust.InstNoOp(name=f"WSPILL-{ctr}")
                    ctr += 1
                    nop.engine = inst.engine
                    nop.sync_info = bass_rust.SyncInfo(on_wait=chunk, on_update=[])
                    newlist.append(nop)
                inst.sync_info = bass_rust.SyncInfo(
                    on_wait=keep, on_update=list(si.on_update or [])
                )
                dirty = True
            newlist.append(inst)
        if dirty:
            bb.instructions = newlist
    return ctr


def _bcast_dram(dram_ap, parts):
    """Partition-stride-0 DMA source view of a 1-D DRAM tensor: [n] -> [parts, n]."""
    return bass.AP(
        tensor=dram_ap.tensor,
        offset=dram_ap.offset,
        ap=[[0, parts]] + list(dram_ap.ap),
    )


def _bcast_row(row_ap, parts):
    """SBUF one-row AP -> [parts, n] via a free-dim stride-0 middle dim."""
    ap = list(row_ap.ap)
    return bass.AP(
        tensor=row_ap.tensor,
        offset=row_ap.offset,
        ap=[ap[0], [0, parts]] + ap[1:],
    )


def build_nc(split_waits=True, max_phase=6, loop_n=None, unroll=1, probe=None):
    nc = bass.Bass(target_bir_lowering=False)

    xT_e = nc.declare_dram_parameter("xT", [128, KT, P], BF, isOutput=False)
    wqk_e = nc.declare_dram_parameter("wqk", [128, KT, 2 * D], BF, isOutput=False)
    wv_e = nc.declare_dram_parameter("wv", [128, KT, D], BF, isOutput=False)
    wp_e = nc.declare_dram_parameter("wp", [128, KT, D], BF, isOutput=False)
    bqk_e = nc.declare_dram_parameter("bqk", [128, 2 * D // 128], F32, isOutput=False)
    bv_e = nc.declare_dram_parameter("bv", [D], F32, isOutput=False)
    bp_e = nc.declare_dram_parameter("bp", [D], F32, isOutput=False)
    out_e = nc.declare_dram_parameter("out", [P, D], BF, isOutput=True)
    waT_e = nc.declare_dram_parameter("waT", [D, P], BF, isOutput=True)

    EXP = mybir.ActivationFunctionType.Exp

    with tile.TileContext(nc) as tc, ExitStack() as ctx:
        if loop_n is not None:
            ctx.enter_context(tc.For_i(0, loop_n, 1))
        const = ctx.enter_context(tc.tile_pool(name="const", bufs=1))
        qkp = ctx.enter_context(tc.tile_pool(name="qkp", bufs=1))
        vxp = ctx.enter_context(tc.tile_pool(name="vxp", bufs=1))
        wtp = ctx.enter_context(tc.tile_pool(name="wtp", bufs=1))
        ptp = ctx.enter_context(tc.tile_pool(name="ptp", bufs=12))
        stgp = ctx.enter_context(tc.tile_pool(name="stgp", bufs=1))
        rbp = ctx.enter_context(tc.tile_pool(name="rbp", bufs=1))
        outp = ctx.enter_context(tc.tile_pool(name="outp", bufs=4))
        psum = ctx.enter_context(tc.tile_pool(name="psum", bufs=4, space="PSUM"))

        def alloc_consts():
            # ---- constant loads: DMA order = first-use order, chunked ------
            C = {}
            C["xT"] = const.tile([128, KT, P], BF, tag="xT", name="xT")
            C["wqk"] = const.tile([128, KT, 2 * D], BF, tag="wqk", name="wqk")
            C["wv"] = const.tile([128, KT, D], BF, tag="wv", name="wv")
            C["wp"] = const.tile([128, KT, D], BF, tag="wp", name="wp")
            C["bqk"] = const.tile([128, 2 * D // 128], F32, tag="bqk", name="bqk")
            C["bvb"] = const.tile([128, D], F32, tag="bvb", name="bvb")
            C["bpb"] = const.tile([128, D], F32, tag="bpb", name="bpb")
            for c in range(3):
                nc.sync.dma_start(out=C["xT"][:, 2 * c : 2 * c + 2, :], in_=xT_e[:, 2 * c : 2 * c + 2, :])
                nc.sync.dma_start(out=C["wqk"][:, 2 * c : 2 * c + 2, :], in_=wqk_e[:, 2 * c : 2 * c + 2, :])
            for c in range(2):
                nc.sync.dma_start(out=C["wv"][:, 3 * c : 3 * c + 3, :], in_=wv_e[:, 3 * c : 3 * c + 3, :])
            nc.sync.dma_start(out=C["bqk"], in_=bqk_e[:])
            nc.sync.dma_start(out=C["bvb"], in_=_bcast_dram(bv_e[:], 128))
            for c in range(2):
                nc.sync.dma_start(out=C["wp"][:, 3 * c : 3 * c + 3, :], in_=wp_e[:, 3 * c : 3 * c + 3, :])
            nc.sync.dma_start(out=C["bpb"], in_=_bcast_dram(bp_e[:], 128))
            return C

        def alloc_qkT():
            return [qkp.tile([128, P], BF, tag=f"qkT{m}", name=f"qkT{m}") for m in range(2 * D // 128)]

        def emit_qkT_c(C, qkTl, ms):
            for m in ms:
                ps = psum.tile([128, 1024], F32, tag="ps", name="ps")
                for k in range(KT):
                    for j in range(2):
                        nc.tensor.matmul(
                            ps[:, j * 512 : (j + 1) * 512],
                            lhsT=C["wqk"][:, k, m * 128 : (m + 1) * 128],
                            rhs=C["xT"][:, k, j * 512 : (j + 1) * 512],
                            start=(k == 0),
                            stop=(k == KT - 1),
                        )
                nc.vector.tensor_scalar_add(qkTl[m], ps, C["bqk"][:, m : m + 1])

        Ccur = qkTcur = None
        for _it in range(unroll):
            first = Ccur is None
            if first:
                Ccur = alloc_consts()
            C = Ccur
            xT, wqk, wv, wp = C["xT"], C["wqk"], C["wv"], C["wp"]
            bqk, bvb, bpb = C["bqk"], C["bvb"], C["bpb"]

            if max_phase < 2:
                for qt in range(QT):
                    nc.sync.dma_start(out=out_e[qt * 128 : (qt + 1) * 128, :], in_=xT[:, 0, 0:D])
                for pr in range(6):
                    nc.gpsimd.dma_start(out=waT_e[pr * 128 : (pr + 1) * 128, :], in_=xT[:, 1, :])
                continue

            # ---- phase 1: qT / kT = (w_qk)^T @ x^T  [feature-major] --------
            # qkT[m] rows = features m*128..; m 0..5 -> q, 6..11 -> k.
            # The first two tiles of iteration i+1 are emitted in iteration
            # i's tail (software pipelining); remaining pairs interleave.
            if first:
                qkTcur = alloc_qkT()
            qkT = qkTcur

            def emit_qkT(ms):
                emit_qkT_c(C, qkT, ms)

            first_ms = [0, 6] if (probe is None and max_phase >= 3) else ([] if probe == 'v' else list(range(12)))
            if first:
                emit_qkT(first_ms)

            # ---- phase 2: v natural [seq-major] with ones column ------------
            vext = [vxp.tile([128, H, HD + 1], BF, tag=f"vext{p}", name=f"vext{p}") for p in range(QT)]
            for p in range(QT if probe != 'qkT' else 0):
                nc.vector.memset(vext[p][:, :, HD : HD + 1], 1.0)
                ps = psum.tile([128, 1024], F32, tag="ps", name="ps")
                for k in range(KT):
                    for (c0, cw) in ((0, 512), (512, 256)):
                        nc.tensor.matmul(
                            ps[:, c0 : c0 + cw],
                            lhsT=xT[:, k, p * 128 : (p + 1) * 128],
                            rhs=wv[:, k, c0 : c0 + cw],
                            start=(k == 0),
                            stop=(k == KT - 1),
                        )
                nc.vector.tensor_add(
                    vext[p][:, :, 0:HD],
                    ps[:, 0:D].rearrange("p (h d) -> p h d", d=HD),
                    bvb.rearrange("p (h d) -> p h d", d=HD),
                )

            # ---- phase 3: attention per head pair ---------------------------
            # S^T tiles [k-positions, q]; exp on ScalarE (scale folded); AV
            # lags one k-tile. Per-pair AV accumulators [65,1024] hold the
            # softmax denominator in row 64 (ones column of vext).
            waTp = [wtp.tile([128, P], BF, tag=f"waTp{p}", name=f"waTp{p}") for p in range(H // 2)]
            densA = stgp.tile([8, P], BF, tag="densA", name="densA")
            densB1 = stgp.tile([2, P], BF, tag="densB1", name="densB1")
            densB2 = stgp.tile([2, P], BF, tag="densB2", name="densB2")
            recipA = stgp.tile([8, P], BF, tag="recipA", name="recipA")
            recipB1 = stgp.tile([2, P], BF, tag="recipB1", name="recipB1")
            recipB2 = stgp.tile([2, P], BF, tag="recipB2", name="recipB2")
            rb = [rbp.tile([HD, P], BF, tag=f"rb{h}", name=f"rb{h}") for h in range(H)]
            stg_tiles = {}

            def emit_recip(grp):
                # 1/denominator on DVE (bf16 out), then per-head broadcast.
                # Reciprocal partition offsets must be 0/32-aligned, so each
                # batch lives at partition 0 of its own tile.
                dens, recip, hs = {
                    0: (densA, recipA, range(0, 8)),
                    1: (densB1, recipB1, range(8, 10)),
                    2: (densB2, recipB2, range(10, 12)),
                }[grp]
                with nc.allow_low_precision(reason="softmax reciprocal to bf16"):
                    nc.vector.reciprocal(recip, dens)
                for h in hs:
                    r = recip[h - hs[0] : h - hs[0] + 1, :]
                    nc.sync.dma_start(out=rb[h], in_=_bcast_row(r, HD))

            def emit_norm(heads):
                # waTp[pair][0:64] = stg_even * rb ; odd half staged through a
                # temp then DMA-merged into partitions 64:128
                for h in heads:
                    if h % 2 == 0:
                        nc.vector.tensor_mul(
                            waTp[h // 2][0:HD, :], stg_tiles[h][0:HD, :], rb[h]
                        )
                    else:
                        wt = outp.tile([HD, P], BF, tag="wtmp", name="wtmp")
                        nc.vector.tensor_mul(wt, stg_tiles[h][0:HD, :], rb[h])
                        eng = nc.sync if h >= 10 else nc.gpsimd
                        eng.dma_start(out=waTp[h // 2][HD:128, :], in_=wt)

            def emit_waT(pr):
                nc.gpsimd.dma_start(out=waT_e[pr * 128 : (pr + 1) * 128, :], in_=waTp[pr])

            # Flat (pair, kt) stream: AV lags S/exp by AV_LAG steps and flows
            # ACROSS pair boundaries, so a new pair's S matmuls cover the old
            # pair's accumulator evacuation latency.
            NPAIR = H // 2 if max_phase >= 3 else 0
            AV_LAG = 4
            psav_all = {}
            pts = {}

            def emit_pair_tail(pr):
                heads = (2 * pr, 2 * pr + 1)
                for h in heads:
                    stg = stgp.tile([HD + 1, P], BF, tag=f"stg{h}", name=f"stg{h}")
                    # last pair: odd head's evac on the (now idle) ScalarE so
                    # both heads evacuate in parallel ahead of the tail chain
                    if h == 11:
                        nc.scalar.copy(stg, psav_all[h])
                    else:
                        nc.vector.tensor_copy(stg, psav_all[h])
                    if h < 8:
                        dtile = densA[h : h + 1, :]
                    elif h < 10:
                        dtile = densB1[h - 8 : h - 7, :]
                    else:
                        dtile = densB2[h - 10 : h - 9, :]
                    nc.sync.dma_start(out=dtile, in_=stg[HD : HD + 1, :])
                    stg_tiles[h] = stg
                if max_phase >= 4:
                    if pr == 3:
                        emit_recip(0)
                        emit_norm(range(0, 4))
                    elif pr == 4:
                        emit_recip(1)
                        emit_norm([8, 9, 4, 5, 6, 7])
                        for p2 in range(0, 5):
                            emit_waT(p2)
                    elif pr == 5:
                        emit_recip(2)
                        emit_norm([11, 10])
                        emit_waT(5)

            def emit_av(pr, kt):
                heads = (2 * pr, 2 * pr + 1)
                if kt == 0:
                    for h in heads:
                        psav_all[h] = psum.tile([HD + 1, P], F32, tag="ps", name="ps")
                for h in heads:
                    for j in range(2):
                        nc.tensor.matmul(
                            psav_all[h][:, j * 512 : (j + 1) * 512],
                            lhsT=vext[kt][:, h, :],
                            rhs=pts[(pr, kt, h)][:, j * 512 : (j + 1) * 512],
                            start=(kt == 0),
                            stop=(kt == QT - 1),
                        )
                if kt == QT - 1:
                    emit_pair_tail(pr)

            steps = [(pr, kt) for pr in range(NPAIR) for kt in range(QT)]
            for i, (pr, kt) in enumerate(steps):
                heads = (2 * pr, 2 * pr + 1)
                pss = {h: psum.tile([128, P], F32, tag="ps", name="ps") for h in heads}
                for h in heads:
                    base = (h % 2) * 64
                    for j in range(2):
                        nc.tensor.matmul(
                            pss[h][:, j * 512 : (j + 1) * 512],
                            lhsT=qkT[6 + h // 2][base : base + 64, kt * 128 : (kt + 1) * 128],
                            rhs=qkT[h // 2][base : base + 64, j * 512 : (j + 1) * 512],
                            start=True,
                            stop=True,
                        )
                    pt = ptp.tile([128, P], BF, tag="pt")
                    nc.scalar.activation(pt, pss[h], EXP, scale=SCALE)
                    pts[(pr, kt, h)] = pt
                # mid-pair qkT interleave keeps the PE warm while exp runs
                if kt == 3 and pr + 1 < NPAIR:
                    emit_qkT([pr + 1])
                if kt == 6 and pr + 1 < NPAIR:
                    emit_qkT([6 + pr + 1])
                if i >= AV_LAG:
                    emit_av(*steps[i - AV_LAG])
            for (pr, kt) in steps[len(steps) - AV_LAG :]:
                emit_av(pr, kt)

            # ---- phase 4: proj per q-tile ----------------------------------
            # First 4 q-tiles accumulate pairs 0..4 while the last pair's
            # normalization chain completes, then take pair 5 and evacuate.
            def proj_mm(ps, qt, p, start, stop):
                for (c0, cw) in ((0, 512), (512, 256)):
                    nc.tensor.matmul(
                        ps[:, c0 : c0 + cw],
                        lhsT=waTp[p][:, qt * 128 : (qt + 1) * 128],
                        rhs=wp[:, p, c0 : c0 + cw],
                        start=start,
                        stop=stop,
                    )

            def proj_fin(ps, qt):
                proj_mm(ps, qt, H // 2 - 1, False, True)
                out_sb = outp.tile([128, D], BF, tag="out_sb", name="out_sb")
                nc.vector.tensor_add(out_sb, ps[:, 0:D], bpb)
                nc.gpsimd.dma_start(out=out_e[qt * 128 : (qt + 1) * 128, :], in_=out_sb)

            Cnext = None
            if max_phase >= 6:
                pss_proj = {}
                for qt in range(2):
                    ps = psum.tile([128, 1024], F32, tag="ps", name="ps")
                    for p in range(H // 2 - 1):
                        proj_mm(ps, qt, p, p == 0, False)
                    pss_proj[qt] = ps
                # next iteration's prologue fills the PE while the last
                # pair's normalization chain completes
                if _it + 1 < unroll and probe is None and max_phase >= 6:
                    Cnext = alloc_consts()
                    qkTnext = alloc_qkT()
                    emit_qkT_c(Cnext, qkTnext, [0, 6])
                for qt in range(2):
                    proj_fin(pss_proj[qt], qt)
                for qt in range(2, QT):
                    ps = psum.tile([128, 1024], F32, tag="ps", name="ps")
                    for p in range(H // 2 - 1):
                        proj_mm(ps, qt, p, p == 0, False)
                    proj_fin(ps, qt)
            if Cnext is not None:
                Ccur, qkTcur = Cnext, qkTnext
            else:
                Ccur = qkTcur = None

            if max_phase < 6:
                for qt in range(QT):
                    nc.sync.dma_start(out=out_e[qt * 128 : (qt + 1) * 128, :], in_=qkT[0][:, 0:D])
            if max_phase < 4:
                for pr in range(6):
                    nc.gpsimd.dma_start(out=waT_e[pr * 128 : (pr + 1) * 128, :], in_=qkT[1][:, 0:P])

    if split_waits:
        _split_excess_waits(nc)
    return nc


def make_in_maps(x, w_qkv, b_qkv, w_proj, b_proj):
    """Host-side shard prep: batch element b -> core b; weights replicated."""
    xf = np.asarray(x, dtype=np.float32)
    wqkv = np.asarray(w_qkv, dtype=np.float32)
    bqkv = np.asarray(b_qkv, dtype=np.float32)
    wproj = np.asarray(w_proj, dtype=np.float32)
    bproj = np.asarray(b_proj, dtype=np.float32)

    wqk = np.ascontiguousarray(
        wqkv[:, : 2 * D].reshape(KT, 128, 2 * D).transpose(1, 0, 2)
    ).astype(NP_BF16)
    wv = np.ascontiguousarray(
        wqkv[:, 2 * D :].reshape(KT, 128, D).transpose(1, 0, 2)
    ).astype(NP_BF16)
    wp = np.ascontiguousarray(
        wproj.reshape(KT, 128, D).transpose(1, 0, 2)
    ).astype(NP_BF16)
    bqk = np.ascontiguousarray(bqkv[: 2 * D].reshape(2 * D // 128, 128).T)
    bv = np.ascontiguousarray(bqkv[2 * D :])

    in_maps = []
    for b in range(N_CORES):
        xT = np.ascontiguousarray(
            xf[b].T.reshape(KT, 128, P).transpose(1, 0, 2)
        ).astype(NP_BF16)
        in_maps.append(
            {
                "xT": xT,
                "wqk": wqk,
                "wv": wv,
                "wp": wp,
                "bqk": bqk,
                "bv": bv,
                "bp": bproj,
            }
        )
    return in_maps


_CACHE = {}


def _get_nc():
    if "nc" not in _CACHE:
        _CACHE["nc"] = build_nc()
    return _CACHE["nc"]


def run_once(in_maps, nc=None):
    if nc is None:
        nc = _get_nc()
    return bass2jax.run_bass_via_pjrt(nc, in_maps, n_cores=N_CORES)


def kernel(x, w_qkv, b_qkv, w_proj, b_proj):
    in_maps = make_in_maps(x, w_qkv, b_qkv, w_proj, b_proj)
    results = run_once(in_maps)
    out = np.stack([results[b]["out"] for b in range(N_CORES)]).astype(np.float32)
    wa = np.stack(
        [results[b]["waT"].astype(np.float32).T for b in range(N_CORES)]
    )
    return (out, wa)
